# revision 5
# baseline (speedup 1.0000x reference)
"""Trainium2 Bass kernel for nn_LogMarginalLikelihood (GP log-marginal-likelihood
via batched CG + stochastic Lanczos quadrature).

Self-contained: hardcodes shapes N=8192, T=101 (y + 100 probes), 30 CG
iterations, 8-way column sharding of the (symmetric) kernel matrix.

Device algorithm (per core c, SPMD on 8 NeuronCores):
  - K shard: columns [1024c:1024(c+1)] of K, cast to fp16, resident in SBUF.
  - CG state held transposed: R^T, P^T fp32 [101, 1024] shards.
  - Matvec: Vt^T[:, i] = sum_b Pn_b^T @ K[b-block, i] with fp16 scaled
    natural-layout P blocks stationary, K moving (N=512 -> near-peak PE).
  - Per-column scaling s = sqrt(rs) keeps fp16 in range (K is rank-256 + I,
    so CG converges ~1e-27; unscaled P underflows fp16).
  - pv dot fused (tensor_tensor_reduce) -> AllGather partials -> alpha;
    R update fused (scalar_tensor_tensor); rs = sum R^2 fused -> AllGather;
    P update fused; scaled fp16 cast; 8 PE transposes -> AllGather natural P.
  - Outputs: alpha' = rs/pv_raw history [101,30] and rs history [101,31].
Host: alpha_k = alpha'_k/sqrt(rs_k), beta_k = rs_{k+1}/rs_k,
  y^T K^-1 y = sum_k alpha_k rs_k (CG identity), SLQ logdet via batched eigh.
"""

import numpy as np

N = 8192
T = 101            # 1 solve column (y) + 100 probes
PIT = 30           # CG iterations
NCORES = 8
SH = N // NCORES   # 1024 output rows per core
NB = N // 128      # 64 contraction blocks
NBS = SH // 128    # 8 local blocks

_cached = {}


def _build():
    import concourse.bacc as bacc
    import concourse.bass as bass
    import concourse.tile as tile
    from concourse import mybir

    fp32 = mybir.dt.float32
    fp16 = mybir.dt.float16
    Alu = mybir.AluOpType
    Act = mybir.ActivationFunctionType

    nc = bacc.Bacc(None, target_bir_lowering=False, num_devices=NCORES)

    k_shard = nc.dram_tensor("k_shard", [N, SH], fp16, kind="ExternalInput")
    bt_shard = nc.dram_tensor("bt_shard", [T, SH], fp32, kind="ExternalInput")
    p0_nat = nc.dram_tensor("p0_nat", [N, T], fp16, kind="ExternalInput")
    rs0 = nc.dram_tensor("rs0", [T, 1], fp32, kind="ExternalInput")
    ident_in = nc.dram_tensor("ident", [T, T], fp16, kind="ExternalInput")

    alph_out = nc.dram_tensor("alph_p", [T, PIT], fp32, kind="ExternalOutput")
    rs_out = nc.dram_tensor("rs_h", [T, PIT + 1], fp32, kind="ExternalOutput")

    rg = [list(range(NCORES))]

    with tile.TileContext(nc) as tc:
        with (
            tc.tile_pool(name="kpool", bufs=1) as kpool,
            tc.tile_pool(name="persist", bufs=1) as persist,
            tc.tile_pool(name="state", bufs=2) as state,
            tc.tile_pool(name="work", bufs=2) as work,
            tc.tile_pool(name="small", bufs=2) as small,
            tc.tile_pool(name="vt_ps_pool", bufs=2, space="PSUM") as vt_ps_pool,
            tc.tile_pool(name="tr_ps_pool", bufs=2, space="PSUM") as tr_ps_pool,
            tc.tile_pool(name="dram", bufs=2, space="DRAM") as dram,
        ):
            # ---- one-time loads ----
            ksb = kpool.tile([128, NB, SH], fp16)          # K shard, blocked
            kv = k_shard.rearrange("(b p) i -> p b i", p=128)
            for b in range(NB):
                nc.sync.dma_start(ksb[:, b, :], kv[:, b, :])

            pnat = persist.tile([128, NB, T], fp16)        # scaled natural P
            pv0 = p0_nat.rearrange("(b p) j -> p b j", p=128)
            for c in range(8):
                nc.sync.dma_start(pnat[:, 8 * c:8 * c + 8, :], pv0[:, 8 * c:8 * c + 8, :])

            ident = persist.tile([T, T], fp16)
            nc.sync.dma_start(ident[:], ident_in[:])

            rs_h = persist.tile([T, PIT + 1], fp32)
            nc.sync.dma_start(rs_h[:, 0:1], rs0[:])
            alph_h = persist.tile([T, PIT], fp32)

            RT = state.tile([T, SH], fp32, name="RT0", tag="RT")
            PT = state.tile([T, SH], fp32, name="PT0", tag="PT")
            nc.sync.dma_start(RT[:], bt_shard[:])
            nc.sync.dma_start(PT[:], bt_shard[:])

            for k in range(PIT):
                last = k == PIT - 1
                # ---- matvec: Vt^T [101, 1024] in PSUM ----
                vt_ps = vt_ps_pool.tile([T, 2, 512], fp32)
                for b in range(NB):
                    for t in range(2):
                        nc.tensor.matmul(
                            vt_ps[:, t, :],
                            pnat[:, b, :],
                            ksb[:, b, 512 * t:512 * t + 512],
                            start=(b == 0),
                            stop=(b == NB - 1),
                        )
                vt = work.tile([T, SH], fp32, tag="vt")
                nc.vector.tensor_copy(vt[:], vt_ps[:, :, :].rearrange("p a b -> p (a b)"))

                # ---- pv partial + allgather + reduce ----
                scr = work.tile([T, SH], fp32, tag="scr")
                pv_part = small.tile([T, 1], fp32, tag="pvp")
                nc.vector.tensor_tensor(scr[:], PT[:], vt[:], Alu.mult)
                nc.vector.tensor_reduce(pv_part[:], scr[:], mybir.AxisListType.X, Alu.add)
                ag1_in = dram.tile([T, 1], fp32, tag="ag1i")
                ag1_out = dram.tile([NCORES, T], fp32, tag="ag1o", addr_space="Shared")
                nc.sync.dma_start(ag1_in[:], pv_part[:])
                nc.gpsimd.collective_compute(
                    "AllGather", Alu.bypass, replica_groups=rg,
                    ins=[ag1_in.opt()], outs=[ag1_out.opt()],
                )
                pv_all = small.tile([T, NCORES], fp32, tag="pva")
                nc.sync.dma_start(pv_all[:], ag1_out.rearrange("r p -> p r"))
                pv_raw = small.tile([T, 1], fp32, tag="pvr")
                nc.vector.tensor_reduce(pv_raw[:], pv_all[:], mybir.AxisListType.X, Alu.add)

                # alpha' = rs * (1/pv_raw); store to history
                pvinv = small.tile([T, 1], fp32, tag="pvi")
                nc.vector.reciprocal(pvinv[:], pv_raw[:])
                nc.vector.tensor_tensor(
                    alph_h[:, k:k + 1], rs_h[:, k:k + 1], pvinv[:], Alu.mult
                )
                nalph = small.tile([T, 1], fp32, tag="nal")
                nc.vector.tensor_scalar_mul(nalph[:], alph_h[:, k:k + 1], -1.0)

                # ---- R update + rs ----
                RTn = state.tile([T, SH], fp32, name=f"RT{k + 1}", tag="RT")
                nc.vector.scalar_tensor_tensor(
                    RTn[:], vt[:], nalph[:], RT[:], Alu.mult, Alu.add
                )
                rs_part = small.tile([T, 1], fp32, tag="rsp")
                nc.vector.tensor_tensor(scr[:], RTn[:], RTn[:], Alu.mult)
                nc.vector.tensor_reduce(rs_part[:], scr[:], mybir.AxisListType.X, Alu.add)
                ag2_in = dram.tile([T, 1], fp32, tag="ag2i")
                ag2_out = dram.tile([NCORES, T], fp32, tag="ag2o", addr_space="Shared")
                nc.sync.dma_start(ag2_in[:], rs_part[:])
                nc.gpsimd.collective_compute(
                    "AllGather", Alu.bypass, replica_groups=rg,
                    ins=[ag2_in.opt()], outs=[ag2_out.opt()],
                )
                rs_all = small.tile([T, NCORES], fp32, tag="rsa")
                nc.sync.dma_start(rs_all[:], ag2_out.rearrange("r p -> p r"))
                nc.vector.tensor_reduce(
                    rs_h[:, k + 1:k + 2], rs_all[:], mybir.AxisListType.X, Alu.add
                )
                RT = RTn
                if last:
                    break

                # ---- beta, P update, scaled fp16 cast ----
                rsinv = small.tile([T, 1], fp32, tag="rsi")
                nc.vector.reciprocal(rsinv[:], rs_h[:, k:k + 1])
                beta = small.tile([T, 1], fp32, tag="bet")
                nc.vector.tensor_tensor(beta[:], rs_h[:, k + 1:k + 2], rsinv[:], Alu.mult)
                PTn = state.tile([T, SH], fp32, name=f"PT{k + 1}", tag="PT")
                nc.vector.scalar_tensor_tensor(
                    PTn[:], PT[:], beta[:], RTn[:], Alu.mult, Alu.add
                )
                PT = PTn
                s_new = small.tile([T, 1], fp32, tag="snw")
                nc.scalar.activation(s_new[:], rs_h[:, k + 1:k + 2], Act.Sqrt)
                sinv = small.tile([T, 1], fp32, tag="siv")
                nc.vector.reciprocal(sinv[:], s_new[:])
                pt16 = work.tile([T, SH], fp16, tag="pt16")
                nc.vector.tensor_scalar_mul(pt16[:], PT[:], sinv[:])

                # ---- transpose local shard to natural, allgather ----
                pn_sh = work.tile([128, NBS, T], fp16, tag="pnsh")
                for j in range(NBS):
                    tr_ps = tr_ps_pool.tile([128, T], fp16)
                    nc.tensor.transpose(
                        tr_ps[:], pt16[:, 128 * j:128 * j + 128], ident[:]
                    )
                    nc.vector.tensor_copy(pn_sh[:, j, :], tr_ps[:])
                ag3_in = dram.tile([SH, T], fp16, tag="ag3i")
                ag3_out = dram.tile([N, T], fp16, tag="ag3o", addr_space="Shared")
                nc.sync.dma_start(
                    ag3_in.rearrange("(j p) t -> p j t", p=128), pn_sh[:]
                )
                nc.gpsimd.collective_compute(
                    "AllGather", Alu.bypass, replica_groups=rg,
                    ins=[ag3_in.opt()], outs=[ag3_out.opt()],
                )
                pnat = persist.tile([128, NB, T], fp16, name=f"pnat{k}", tag="pnat_t", bufs=2)
                agv = ag3_out.rearrange("(b p) t -> p b t", p=128)
                for c in range(8):
                    nc.sync.dma_start(
                        pnat[:, 8 * c:8 * c + 8, :], agv[:, 8 * c:8 * c + 8, :]
                    )

            nc.sync.dma_start(alph_out[:], alph_h[:])
            nc.sync.dma_start(rs_out[:], rs_h[:])

    nc.compile()
    return nc


def _get_nc():
    if "nc" not in _cached:
        _cached["nc"] = _build()
    return _cached["nc"]


def kernel(Knn_noise: np.ndarray, y: np.ndarray, Z: np.ndarray) -> np.ndarray:
    from concourse.bass_utils import run_bass_kernel_spmd

    K = np.ascontiguousarray(Knn_noise, dtype=np.float32)
    B = np.concatenate([y.astype(np.float32), Z.astype(np.float32)], axis=1)  # [N, T]
    rs0 = np.sum(B * B, axis=0)                       # [T]
    s0 = np.sqrt(rs0)
    p0 = (B / s0[None, :]).astype(np.float16)
    K16 = K.astype(np.float16)
    BT = np.ascontiguousarray(B.T)                    # [T, N]
    ident = np.eye(T, dtype=np.float16)

    in_maps = []
    for c in range(NCORES):
        in_maps.append({
            "k_shard": np.ascontiguousarray(K16[:, SH * c:SH * (c + 1)]),
            "bt_shard": np.ascontiguousarray(BT[:, SH * c:SH * (c + 1)]),
            "p0_nat": p0,
            "rs0": rs0.reshape(T, 1).astype(np.float32),
            "ident": ident,
        })

    nc = _get_nc()
    _cached["last_in_maps"] = in_maps
    res = run_bass_kernel_spmd(nc, in_maps, core_ids=list(range(NCORES)))
    out0 = res.results[0]
    alph_p = out0["alph_p"].astype(np.float64)        # [T, PIT]
    rs_h = out0["rs_h"].astype(np.float64)            # [T, PIT+1]

    rs_k = rs_h[:, :PIT]                              # [T, PIT]
    alphas = (alph_p / np.sqrt(rs_k)).T               # [PIT, T]
    betas = (rs_h[:, 1:PIT + 1] / rs_k).T             # [PIT, T]

    yKy = float(np.sum(alphas[:, 0] * rs_k.T[:, 0]))

    a = alphas[:, 1:]
    b = betas[:, 1:]
    inv_a = 1.0 / a
    diag = inv_a.copy()
    diag[1:] += b[:-1] / a[:-1]
    off = np.sqrt(np.maximum(b[:-1], 0.0)) / a[:-1]
    Ts = np.zeros((T - 1, PIT, PIT))
    idx = np.arange(PIT)
    Ts[:, idx, idx] = diag.T
    Ts[:, idx[:-1], idx[1:]] = off.T
    Ts[:, idx[1:], idx[:-1]] = off.T
    lam, V = np.linalg.eigh(Ts)
    lam = np.maximum(lam, 1e-12)
    quad = np.sum(V[:, 0, :] ** 2 * np.log(lam), axis=1)
    log_det = N * float(np.mean(quad))

    out = -0.5 * yKy - 0.5 * log_det - N * 0.5 * np.log(2.0 * np.pi)
    return np.array([[out]], dtype=np.float32)
